# revision 5
# baseline (speedup 1.0000x reference)
"""Trainium2 Bass kernel for nn_Column1_20298015441326 (topk_masking).

Reference computation (per branch r of RF=512, fully independent):
  pot[r,t,k] = sum_l rec_field[t,0,r,l] * W[r,k,0,l]      (T=32, K=32, L=2048)
  thr = pot * (pot > 20);  spikes = sign(thr)
  kWTA top-4 winner mask per branch (SpykeTorch get_k_winners semantics,
  ties broken by lower feature index), out = spikes * mask, -> (T,1,K,RF).

Sharding: branch axis across 8 cores (64 branches/core), no cross-core comms.

v2 (precision-split inputs, DMA-roofline focused):
  The kernel is memory-bound; fp32 traffic was 33.8 MB/core (~78 us at the
  435 GB/s DMA cap). W is sent as fp16 (validated end-to-end: rel err 0.0096
  vs the 2e-2 budget) and x as an exact-ish fp16 pair xh + xl/64 (keeps x at
  ~2^-22 relative, same bytes as fp32), cutting traffic to 25.2 MB/core and
  making every matmul a full-rate 16-bit pass (fp32 matmuls cost 4 cycles/row
  on the PE; fp16 costs 1).

Per-core device layout:
  branches b = g*4 + rs  (g in [0,16) groups, rs in [0,4) col-tiles)
  xw dram (128, G*6144) fp16: per group block of 6144 cols = [xh|xl|w],
  each 2048 = rs*512 + c*32 + (t|k), partition p = contraction lane
  (l = c*128 + p). Transfers slice contiguous column ranges so every DMA
  descriptor is a 12-24 KB contiguous run per partition (hits the 435 GB/s
  aggregate cap at ~610ns/16KB/engine). Taper 1,1,2,...,2,1,1 groups: small
  head so the PE starts early, small tail so the last group computes early.
  PE per (g,rs,c): two fp16 matmuls (hi, lo) accumulate into separate PSUM
  tiles (8 persistent tiles, one bank each; no recycling).
  pot = ps_hi + ps_lo/64 on DVE directly from PSUM (no scalar.copy -> no ACT
  table-load DMAs competing with the input stream).
  Post-processing on DVE as before, but the transpose/top-4/mask/apply chain
  runs twice (groups 0-7 after g=7, groups 8-15 at the end) so only half the
  chain sits on the critical tail; output DMAs go on the gpsimd queue to
  avoid queueing behind input descriptors on the sync queue.
"""

import numpy as np

import concourse.bacc as bacc
import concourse.mybir as mybir
from concourse import bass_utils
from concourse.tile import TileContext

T = 32
K = 32
RF = 512
L = 2048
TH = 20.0
NCORES = 8
G = 16          # branch groups per core
RS = 4          # branches per group (PE col tiles)
CH = 16         # contraction chunks of 128
GB = 3 * 2048   # xw cols per group: [xh | xl | w]
LO = 64.0       # xl scale
TRANSFERS = [(0, 1), (1, 2), (2, 4), (4, 6), (6, 8), (8, 10), (10, 12),
             (12, 14), (14, 15), (15, 16)]
DUMMY_AFTER = 4  # PE stream waits for this transfer (delayed warm start)
F32 = mybir.dt.float32
F16 = mybir.dt.float16
Ax = mybir.AxisListType
Op = mybir.AluOpType

_CACHE = {}


def build():
    """Build + compile the per-core Bass module (SPMD: same program, 8 cores)."""
    nc = bacc.Bacc("TRN2", target_bir_lowering=False, debug=False, num_devices=NCORES)
    xw = nc.dram_tensor("xw", (128, G * GB), F16, kind="ExternalInput")
    iota_d = nc.dram_tensor("iota_t", (128, T), F32, kind="ExternalInput")
    out = nc.dram_tensor("out", (128, G * T), F32, kind="ExternalOutput")

    with TileContext(nc) as tc:
        with tc.tile_pool(name="io", bufs=7) as io, \
             tc.tile_pool(name="psp", bufs=1, space="PSUM") as psp, \
             tc.tile_pool(name="wk", bufs=1) as wk:
            iota_sb = wk.tile([128, T], F32)
            nc.gpsimd.dma_start(out=iota_sb[:], in_=iota_d[:, :])
            zeros = wk.tile([128, K], F32)
            nc.vector.memset(zeros[:], 0.0)

            pot = wk.tile([128, G * T], F32)
            gt = wk.tile([128, G * T], F32)
            thr = wk.tile([128, G * T], F32)
            sel = wk.tile([128, G * T], F32)
            sel2 = wk.tile([128, G * T], F32)
            # packed (128, 96): [cnt | pad | vals | pad | rowmax | pad] (16 each)
            packed = wk.tile([128, 96], F32)
            nc.vector.memset(packed[:], 0.0)
            first = wk.tile([128, G], F32)
            has = wk.tile([128, G], F32)

            def stage_a(glo, ghi):
                """fire + per-feature stats for groups [glo, ghi)."""
                gn = ghi - glo
                fs = slice(glo * T, ghi * T)
                g3 = gt[:, fs].rearrange("p (g t) -> p g t", t=T)
                t3 = thr[:, fs].rearrange("p (g t) -> p g t", t=T)
                s3 = sel[:, fs].rearrange("p (g t) -> p g t", t=T)
                s23 = sel2[:, fs].rearrange("p (g t) -> p g t", t=T)
                gsl = slice(glo, ghi)
                nc.vector.tensor_scalar(
                    out=gt[:, fs], in0=pot[:, fs], scalar1=TH, scalar2=None,
                    op0=Op.is_gt)
                nc.vector.tensor_tensor(
                    out=thr[:, fs], in0=pot[:, fs], in1=gt[:, fs], op=Op.mult)
                cnt = packed[:, glo:ghi]
                nc.vector.reduce_sum(out=cnt, in_=g3, axis=Ax.X)
                # first spike time: min(32 - cnt, 31)
                nc.vector.tensor_scalar(
                    out=first[:, gsl], in0=cnt, scalar1=32.0, scalar2=-1.0,
                    op0=Op.subtract, op1=Op.mult)
                nc.vector.tensor_scalar(
                    out=first[:, gsl], in0=first[:, gsl], scalar1=31.0,
                    scalar2=None, op0=Op.min)
                # vals_at_first = sum_t thr * (iota_t == first)
                nc.vector.tensor_tensor(
                    out=s3,
                    in0=iota_sb[:, None, :].to_broadcast([128, gn, T]),
                    in1=first[:, gsl, None].to_broadcast([128, gn, T]),
                    op=Op.is_equal)
                nc.vector.tensor_tensor(out=s23, in0=s3, in1=t3, op=Op.mult)
                vals = packed[:, 32 + glo:32 + ghi]
                nc.vector.reduce_sum(out=vals, in_=s23, axis=Ax.X)
                # rowmax = vals * (cnt > 0)
                nc.vector.tensor_scalar(
                    out=has[:, gsl], in0=cnt, scalar1=0.0, scalar2=None,
                    op0=Op.is_gt)
                nc.vector.tensor_tensor(
                    out=packed[:, 64 + glo:64 + ghi], in0=vals, in1=has[:, gsl],
                    op=Op.mult)

            def chain(tag, glo, ghi):
                """transpose + per-branch top-4 + mask + apply + store for
                groups [glo, ghi). Runs on the full packed tile (stale
                columns produce garbage in unused partitions); only the
                [glo, ghi) output columns are written out."""
                # 32x32 block transpose: -> [p=(rs,g), free=k] per 32-block
                tp = wk.tile([128, 96], F32, name=f"tp{tag}")
                nc.vector.transpose(out=tp[:], in_=packed[:])
                cntT = tp[:, 0:32]
                valsT = tp[:, 32:64]
                rowmaxT = tp[:, 64:96]

                # per-branch v = 32 * max_k rowmax;  total = cnt * (vals + v)
                vmax = wk.tile([128, 1], F32, name=f"vmax{tag}")
                nc.vector.reduce_max(out=vmax[:], in_=rowmaxT, axis=Ax.X)
                v32 = wk.tile([128, 1], F32, name=f"v32{tag}")
                nc.vector.tensor_scalar(
                    out=v32[:], in0=vmax[:], scalar1=32.0, scalar2=None,
                    op0=Op.mult)
                tot2 = wk.tile([128, K], F32, name=f"tot2{tag}")
                nc.vector.scalar_tensor_tensor(
                    out=tot2[:], in0=valsT, scalar=v32[:], in1=cntT,
                    op0=Op.add, op1=Op.mult)

                # top-4 with stable (lower index first) tie-break:
                # m4c = max(4th largest, tiny); keep (tot > m4c) plus the first
                # (4 - #gt) entries equal to m4c. The tiny clamp makes the m4=0
                # case (fewer than 4 positive totals) select exactly the
                # positives, since no total equals the clamp value.
                m8 = wk.tile([128, 8], F32, name=f"m8{tag}")
                nc.vector.max(out=m8[:], in_=tot2[:])
                m4c = wk.tile([128, 1], F32, name=f"m4c{tag}")
                nc.vector.tensor_scalar(
                    out=m4c[:], in0=m8[:, 3:4], scalar1=1e-30, scalar2=None,
                    op0=Op.max)
                sg = wk.tile([128, K], F32, name=f"sg{tag}")
                eq = wk.tile([128, K], F32, name=f"eq{tag}")
                nc.vector.tensor_scalar(
                    out=sg[:], in0=tot2[:], scalar1=m4c[:], scalar2=None,
                    op0=Op.is_gt)
                nc.vector.tensor_scalar(
                    out=eq[:], in0=tot2[:], scalar1=m4c[:], scalar2=None,
                    op0=Op.is_equal)
                ng = wk.tile([128, 1], F32, name=f"ng{tag}")
                nc.vector.reduce_sum(out=ng[:], in_=sg[:], axis=Ax.X)
                need = wk.tile([128, 1], F32, name=f"need{tag}")
                nc.vector.tensor_scalar(
                    out=need[:], in0=ng[:], scalar1=4.0, scalar2=-1.0,
                    op0=Op.subtract, op1=Op.mult)
                incl = wk.tile([128, K], F32, name=f"incl{tag}")
                nc.vector.tensor_tensor_scan(
                    out=incl[:], data0=eq[:], data1=zeros[:], initial=0.0,
                    op0=Op.add, op1=Op.add)
                # eq-element selected iff inclusive-rank <= need
                seleq = wk.tile([128, K], F32, name=f"seleq{tag}")
                nc.vector.tensor_scalar(
                    out=seleq[:], in0=incl[:], scalar1=need[:], scalar2=None,
                    op0=Op.is_le)
                eqs = wk.tile([128, K], F32, name=f"eqs{tag}")
                nc.vector.tensor_tensor(
                    out=eqs[:], in0=eq[:], in1=seleq[:], op=Op.mult)
                maskT = wk.tile([128, K], F32, name=f"maskT{tag}")
                nc.vector.tensor_tensor(
                    out=maskT[:], in0=sg[:], in1=eqs[:], op=Op.add)

                # transpose mask back to [p=(rs,k), free=g], apply, store
                maskA = wk.tile([128, K], F32, name=f"maskA{tag}")
                nc.vector.transpose(out=maskA[:], in_=maskT[:])
                gn = ghi - glo
                fs = slice(glo * T, ghi * T)
                outt = wk.tile([128, gn * T], F32, name=f"outt{tag}")
                o3 = outt[:].rearrange("p (g t) -> p g t", t=T)
                g3 = gt[:, fs].rearrange("p (g t) -> p g t", t=T)
                nc.vector.tensor_tensor(
                    out=o3, in0=g3,
                    in1=maskA[:, glo:ghi, None].to_broadcast([128, gn, T]),
                    op=Op.mult)
                nc.gpsimd.dma_start(out=out[:, fs], in_=outt[:])

            # 8 persistent PSUM tiles (one bank each): hi/lo accumulators for
            # 4-group block g//4, column slice (g%4)*32. No slot recycling ->
            # no release waits on the PE chain. Blocks 0-2 are combined
            # (PSUM->SBUF) 128 cols at a time (fewer PSUM-read stalls); block
            # 3 per group so the last groups' post-processing isn't deferred.
            ph4 = [psp.tile([128, 4 * T], F32, tag=f"ph{j}", name=f"ph{j}")
                   for j in range(4)]
            pl4 = [psp.tile([128, 4 * T], F32, tag=f"pl{j}", name=f"pl{j}")
                   for j in range(4)]

            def combine(ph, pl, c0, c1, g0):
                """pot[g0*T ...] = ps_hi + ps_lo/64 for PSUM cols [c0, c1)
                (one PSUM operand per DVE op)."""
                ps = slice(g0 * T, g0 * T + (c1 - c0))
                nc.vector.tensor_scalar(
                    out=pot[:, ps], in0=pl[:, c0:c1],
                    scalar1=1.0 / LO, scalar2=None, op0=Op.mult)
                nc.vector.tensor_tensor(
                    out=pot[:, ps], in0=pot[:, ps], in1=ph[:, c0:c1], op=Op.add)

            # Issue every input transfer up front (sync queue, in order;
            # pool-slot recycling gates the last three at runtime).
            tiles = []
            for b0, b1 in TRANSFERS:
                xwt = io.tile([128, 2 * GB], F16, tag="xw")
                nc.sync.dma_start(
                    out=xwt[:, :(b1 - b0) * GB],
                    in_=xw[:, b0 * GB:b1 * GB])
                tiles.append(xwt)

            # Delayed PE start: a 1x1 dummy matmul that depends on transfer
            # DUMMY_AFTER makes the whole (in-order) PE stream wait until
            # ~half the input has landed, then run as one continuous warm
            # burst that drains the backlog and finishes with the stream --
            # instead of trickling along with the DMA in short bursts that
            # keep HAM re-throttling the PE clock to 1.2 GHz.
            nc.tensor.matmul(
                out=ph4[3][0:1, 0:1],
                lhsT=tiles[DUMMY_AFTER][:, 0:1],
                rhs=tiles[DUMMY_AFTER][:, 0:1],
                start=True, stop=True)

            for (b0, b1), xwt in zip(TRANSFERS, tiles):
                for gb in range(b1 - b0):
                    g = b0 + gb
                    ph = ph4[g // 4]
                    pl = pl4[g // 4]
                    cs = (g % 4) * T
                    base = gb * GB
                    for c in range(CH):
                        for rs in range(RS):
                            xo = base + rs * 512 + c * 32
                            wo = base + 2 * 2048 + rs * 512 + c * 32
                            nc.tensor.matmul(
                                out=ph[rs * 32:(rs + 1) * 32, cs:cs + T],
                                lhsT=xwt[:, wo:wo + K],
                                rhs=xwt[:, xo:xo + T],
                                start=(c == 0),
                                stop=(c == CH - 1),
                                tile_position=(0, rs * 32),
                            )
                            nc.tensor.matmul(
                                out=pl[rs * 32:(rs + 1) * 32, cs:cs + T],
                                lhsT=xwt[:, wo:wo + K],
                                rhs=xwt[:, 2048 + xo:2048 + xo + T],
                                start=(c == 0),
                                stop=(c == CH - 1),
                                tile_position=(0, rs * 32),
                            )
                    if g >= 12:
                        combine(ph, pl, cs, cs + T, g)
                        stage_a(g, g + 1)
                    elif (g + 1) % 4 == 0:
                        combine(ph, pl, 0, 4 * T, g - 3)
                        stage_a(g - 3, g + 1)
                    if g == 7:
                        chain(0, 0, G // 2)
                    elif g == 15:
                        chain(1, G // 2, G)

    nc.compile()
    return nc


def prep_inputs(rec_field, W):
    """Host-side relayout into the per-core packed fp16 DMA layout."""
    rec_field = np.asarray(rec_field, dtype=np.float32)
    W = np.asarray(W, dtype=np.float32)
    xr = rec_field[:, 0].transpose(1, 2, 0)            # (RF, L, T) f32
    xh = xr.astype(np.float16)
    xl = ((xr - xh.astype(np.float32)) * LO).astype(np.float16)
    wr = W[:, :, 0].transpose(0, 2, 1).astype(np.float16)   # (RF, L, K)

    def lay(a, J):
        # (RF, L, J) -> (d, p, g, rs*ch*J) with l = c*128 + p
        a6 = a.reshape(NCORES, G, RS, CH, 128, J)
        return a6.transpose(0, 4, 1, 2, 3, 5).reshape(NCORES, 128, G, RS * CH * J)

    H = lay(xh, T)
    Lo = lay(xl, T)
    Wl = lay(wr, K)
    blk = np.concatenate([H, Lo, Wl], axis=3)          # (d, p, g, 3*2048)
    return np.ascontiguousarray(blk.reshape(NCORES, 128, G * GB))


def make_in_maps(rec_field, W):
    xwh = prep_inputs(rec_field, W)
    iota = np.ascontiguousarray(
        np.tile(np.arange(T, dtype=np.float32), (128, 1)))
    return [{"iota_t": iota, "xw": xwh[d]} for d in range(NCORES)]


def assemble_output(results):
    """results: per-core dicts with 'out' (128, 512) -> full (T,1,K,RF)."""
    out_full = np.zeros((T, 1, K, RF), np.float32)
    for d in range(NCORES):
        o = np.asarray(results[d]["out"]).reshape(RS, K, G, T)
        o = o.transpose(3, 1, 2, 0).reshape(T, K, G * RS)   # (t, k, b=g*4+rs)
        out_full[:, 0, :, d * (G * RS):(d + 1) * (G * RS)] = o
    return out_full


def get_nc():
    if "nc" not in _CACHE:
        _CACHE["nc"] = build()
    return _CACHE["nc"]


def kernel(rec_field, W, reward=None, **_unused):
    nc = get_nc()
    in_maps = make_in_maps(rec_field, W)
    res = bass_utils.run_bass_kernel_spmd(nc, in_maps, core_ids=list(range(NCORES)))
    return assemble_output(res.results)


# revision 6
# speedup vs baseline: 1.1161x; 1.1161x over previous
"""Trainium2 Bass kernel for nn_Column1_20298015441326 (topk_masking).

Reference computation (per branch r of RF=512, fully independent):
  pot[r,t,k] = sum_l rec_field[t,0,r,l] * W[r,k,0,l]      (T=32, K=32, L=2048)
  thr = pot * (pot > 20);  spikes = sign(thr)
  kWTA top-4 winner mask per branch (SpykeTorch get_k_winners semantics,
  ties broken by lower feature index), out = spikes * mask, -> (T,1,K,RF).

Sharding: branch axis across 8 cores (64 branches/core), no cross-core comms.

v2 (precision-split inputs, DMA-roofline focused):
  The kernel is memory-bound; fp32 traffic was 33.8 MB/core (~78 us at the
  435 GB/s DMA cap). W is sent as fp16 (validated end-to-end: rel err 0.0096
  vs the 2e-2 budget) and x as an exact-ish fp16 pair xh + xl/64 (keeps x at
  ~2^-22 relative, same bytes as fp32), cutting traffic to 25.2 MB/core and
  making every matmul a full-rate 16-bit pass (fp32 matmuls cost 4 cycles/row
  on the PE; fp16 costs 1).

Per-core device layout:
  branches b = g*4 + rs  (g in [0,16) groups, rs in [0,4) col-tiles)
  xw dram (128, G*6144) fp16: per group block of 6144 cols = [xh|xl|w],
  each 2048 = rs*512 + c*32 + (t|k), partition p = contraction lane
  (l = c*128 + p). Transfers slice contiguous column ranges so every DMA
  descriptor is a 12-24 KB contiguous run per partition (hits the 435 GB/s
  aggregate cap at ~610ns/16KB/engine). Taper 1,1,2,...,2,1,1 groups: small
  head so the PE starts early, small tail so the last group computes early.
  PE per (g,rs,c): two fp16 matmuls (hi, lo) accumulate into separate PSUM
  tiles (8 persistent tiles, one bank each; no recycling).
  pot = ps_hi + ps_lo/64 on DVE directly from PSUM (no scalar.copy -> no ACT
  table-load DMAs competing with the input stream).
  Post-processing on DVE as before, but the transpose/top-4/mask/apply chain
  runs twice (groups 0-7 after g=7, groups 8-15 at the end) so only half the
  chain sits on the critical tail; output DMAs go on the gpsimd queue to
  avoid queueing behind input descriptors on the sync queue.
"""

import numpy as np

import concourse.bacc as bacc
import concourse.mybir as mybir
from concourse import bass_utils
from concourse.tile import TileContext

T = 32
K = 32
RF = 512
L = 2048
TH = 20.0
NCORES = 8
G = 16          # branch groups per core
RS = 4          # branches per group (PE col tiles)
CH = 16         # contraction chunks of 128
GB = 3 * 2048   # xw cols per group: [xh | xl | w]
LO = 64.0       # xl scale
TRANSFERS = [(0, 1), (1, 2), (2, 4), (4, 6), (6, 8), (8, 10), (10, 12),
             (12, 14), (14, 15), (15, 16)]
DUMMY_AFTER = 4  # PE stream waits for this transfer (delayed warm start)
F32 = mybir.dt.float32
F16 = mybir.dt.float16
Ax = mybir.AxisListType
Op = mybir.AluOpType

_CACHE = {}


def build():
    """Build + compile the per-core Bass module (SPMD: same program, 8 cores)."""
    nc = bacc.Bacc("TRN2", target_bir_lowering=False, debug=False, num_devices=NCORES)
    xw = nc.dram_tensor("xw", (128, G * GB), F16, kind="ExternalInput")
    iota_d = nc.dram_tensor("iota_t", (128, T), F32, kind="ExternalInput")
    out = nc.dram_tensor("out", (128, G * T), F32, kind="ExternalOutput")

    with TileContext(nc) as tc:
        with tc.tile_pool(name="io", bufs=7) as io, \
             tc.tile_pool(name="psp", bufs=1, space="PSUM") as psp, \
             tc.tile_pool(name="wk", bufs=1) as wk:
            iota_sb = wk.tile([128, T], F32)
            nc.gpsimd.dma_start(out=iota_sb[:], in_=iota_d[:, :])
            zeros = wk.tile([128, K], F32)
            nc.vector.memset(zeros[:], 0.0)

            pot = wk.tile([128, G * T], F32)
            gt = wk.tile([128, G * T], F32)
            thr = wk.tile([128, G * T], F32)
            sel = wk.tile([128, G * T], F32)
            sel2 = wk.tile([128, G * T], F32)
            # packed (128, 96): [cnt | pad | vals | pad | rowmax | pad] (16 each)
            packed = wk.tile([128, 96], F32)
            nc.vector.memset(packed[:], 0.0)
            first = wk.tile([128, G], F32)
            has = wk.tile([128, G], F32)

            def stage_a(glo, ghi):
                """fire + per-feature stats for groups [glo, ghi)."""
                gn = ghi - glo
                fs = slice(glo * T, ghi * T)
                g3 = gt[:, fs].rearrange("p (g t) -> p g t", t=T)
                t3 = thr[:, fs].rearrange("p (g t) -> p g t", t=T)
                s3 = sel[:, fs].rearrange("p (g t) -> p g t", t=T)
                s23 = sel2[:, fs].rearrange("p (g t) -> p g t", t=T)
                gsl = slice(glo, ghi)
                nc.vector.tensor_scalar(
                    out=gt[:, fs], in0=pot[:, fs], scalar1=TH, scalar2=None,
                    op0=Op.is_gt)
                nc.vector.tensor_tensor(
                    out=thr[:, fs], in0=pot[:, fs], in1=gt[:, fs], op=Op.mult)
                cnt = packed[:, glo:ghi]
                nc.vector.reduce_sum(out=cnt, in_=g3, axis=Ax.X)
                # first spike time: min(32 - cnt, 31)
                nc.vector.tensor_scalar(
                    out=first[:, gsl], in0=cnt, scalar1=32.0, scalar2=-1.0,
                    op0=Op.subtract, op1=Op.mult)
                nc.vector.tensor_scalar(
                    out=first[:, gsl], in0=first[:, gsl], scalar1=31.0,
                    scalar2=None, op0=Op.min)
                # vals_at_first = sum_t thr * (iota_t == first)
                nc.vector.tensor_tensor(
                    out=s3,
                    in0=iota_sb[:, None, :].to_broadcast([128, gn, T]),
                    in1=first[:, gsl, None].to_broadcast([128, gn, T]),
                    op=Op.is_equal)
                nc.vector.tensor_tensor(out=s23, in0=s3, in1=t3, op=Op.mult)
                vals = packed[:, 32 + glo:32 + ghi]
                nc.vector.reduce_sum(out=vals, in_=s23, axis=Ax.X)
                # rowmax = vals * (cnt > 0)
                nc.vector.tensor_scalar(
                    out=has[:, gsl], in0=cnt, scalar1=0.0, scalar2=None,
                    op0=Op.is_gt)
                nc.vector.tensor_tensor(
                    out=packed[:, 64 + glo:64 + ghi], in0=vals, in1=has[:, gsl],
                    op=Op.mult)

            def chain(tag, glo, ghi):
                """transpose + per-branch top-4 + mask + apply + store for
                groups [glo, ghi). Runs on the full packed tile (stale
                columns produce garbage in unused partitions); only the
                [glo, ghi) output columns are written out."""
                # 32x32 block transpose: -> [p=(rs,g), free=k] per 32-block
                tp = wk.tile([128, 96], F32, name=f"tp{tag}")
                nc.vector.transpose(out=tp[:], in_=packed[:])
                cntT = tp[:, 0:32]
                valsT = tp[:, 32:64]
                rowmaxT = tp[:, 64:96]

                # per-branch v = 32 * max_k rowmax;  total = cnt * (vals + v)
                vmax = wk.tile([128, 1], F32, name=f"vmax{tag}")
                nc.vector.reduce_max(out=vmax[:], in_=rowmaxT, axis=Ax.X)
                v32 = wk.tile([128, 1], F32, name=f"v32{tag}")
                nc.vector.tensor_scalar(
                    out=v32[:], in0=vmax[:], scalar1=32.0, scalar2=None,
                    op0=Op.mult)
                tot2 = wk.tile([128, K], F32, name=f"tot2{tag}")
                nc.vector.scalar_tensor_tensor(
                    out=tot2[:], in0=valsT, scalar=v32[:], in1=cntT,
                    op0=Op.add, op1=Op.mult)

                # top-4 with stable (lower index first) tie-break:
                # m4c = max(4th largest, tiny); keep (tot > m4c) plus the first
                # (4 - #gt) entries equal to m4c. The tiny clamp makes the m4=0
                # case (fewer than 4 positive totals) select exactly the
                # positives, since no total equals the clamp value.
                m8 = wk.tile([128, 8], F32, name=f"m8{tag}")
                nc.vector.max(out=m8[:], in_=tot2[:])
                m4c = wk.tile([128, 1], F32, name=f"m4c{tag}")
                nc.vector.tensor_scalar(
                    out=m4c[:], in0=m8[:, 3:4], scalar1=1e-30, scalar2=None,
                    op0=Op.max)
                sg = wk.tile([128, K], F32, name=f"sg{tag}")
                eq = wk.tile([128, K], F32, name=f"eq{tag}")
                nc.vector.tensor_scalar(
                    out=sg[:], in0=tot2[:], scalar1=m4c[:], scalar2=None,
                    op0=Op.is_gt)
                nc.vector.tensor_scalar(
                    out=eq[:], in0=tot2[:], scalar1=m4c[:], scalar2=None,
                    op0=Op.is_equal)
                ng = wk.tile([128, 1], F32, name=f"ng{tag}")
                nc.vector.reduce_sum(out=ng[:], in_=sg[:], axis=Ax.X)
                need = wk.tile([128, 1], F32, name=f"need{tag}")
                nc.vector.tensor_scalar(
                    out=need[:], in0=ng[:], scalar1=4.0, scalar2=-1.0,
                    op0=Op.subtract, op1=Op.mult)
                incl = wk.tile([128, K], F32, name=f"incl{tag}")
                nc.vector.tensor_tensor_scan(
                    out=incl[:], data0=eq[:], data1=zeros[:], initial=0.0,
                    op0=Op.add, op1=Op.add)
                # eq-element selected iff inclusive-rank <= need
                seleq = wk.tile([128, K], F32, name=f"seleq{tag}")
                nc.vector.tensor_scalar(
                    out=seleq[:], in0=incl[:], scalar1=need[:], scalar2=None,
                    op0=Op.is_le)
                eqs = wk.tile([128, K], F32, name=f"eqs{tag}")
                nc.vector.tensor_tensor(
                    out=eqs[:], in0=eq[:], in1=seleq[:], op=Op.mult)
                maskT = wk.tile([128, K], F32, name=f"maskT{tag}")
                nc.vector.tensor_tensor(
                    out=maskT[:], in0=sg[:], in1=eqs[:], op=Op.add)

                # transpose mask back to [p=(rs,k), free=g], apply, store
                maskA = wk.tile([128, K], F32, name=f"maskA{tag}")
                nc.vector.transpose(out=maskA[:], in_=maskT[:])
                gn = ghi - glo
                fs = slice(glo * T, ghi * T)
                outt = wk.tile([128, gn * T], F32, name=f"outt{tag}")
                o3 = outt[:].rearrange("p (g t) -> p g t", t=T)
                g3 = gt[:, fs].rearrange("p (g t) -> p g t", t=T)
                nc.vector.tensor_tensor(
                    out=o3, in0=g3,
                    in1=maskA[:, glo:ghi, None].to_broadcast([128, gn, T]),
                    op=Op.mult)
                nc.gpsimd.dma_start(out=out[:, fs], in_=outt[:])

            # 8 persistent PSUM tiles (one bank each): hi/lo accumulators for
            # group g%4 at column slice (g//4)*32 (consecutive groups rotate
            # across banks so a group's combine has 4 groups of slack before
            # its bank columns are needed again). No slot recycling -> no
            # release waits on the PE chain.
            ph4 = [psp.tile([128, 4 * T], F32, tag=f"ph{j}", name=f"ph{j}")
                   for j in range(4)]
            pl4 = [psp.tile([128, 4 * T], F32, tag=f"pl{j}", name=f"pl{j}")
                   for j in range(4)]

            # Issue every input transfer up front (sync queue, in order;
            # pool-slot recycling gates the last three at runtime).
            tiles = []
            for b0, b1 in TRANSFERS:
                xwt = io.tile([128, 2 * GB], F16, tag="xw")
                nc.sync.dma_start(
                    out=xwt[:, :(b1 - b0) * GB],
                    in_=xw[:, b0 * GB:b1 * GB])
                tiles.append(xwt)

            # Delayed PE start: a 1x1 dummy matmul that reads transfer
            # DUMMY_AFTER's tile and writes inside group 0's first PSUM
            # output slice (WAW dependency group 0's start=True matmul
            # overwrites, so the scheduler cannot hoist the real stream
            # above it). This holds the whole in-order PE stream until
            # ~half the input has landed, then runs it as one continuous
            # warm burst that drains the backlog and finishes with the
            # stream -- instead of trickling along with the DMA in short
            # bursts that keep HAM re-throttling the PE clock to 1.2 GHz.
            nc.tensor.matmul(
                out=ph4[0][0:1, 0:1],
                lhsT=tiles[DUMMY_AFTER][:, 0:1],
                rhs=tiles[DUMMY_AFTER][:, 0:1],
                start=True, stop=True)

            for (b0, b1), xwt in zip(TRANSFERS, tiles):
                for gb in range(b1 - b0):
                    g = b0 + gb
                    ph = ph4[g % 4]
                    pl = pl4[g % 4]
                    cs = (g // 4) * T
                    base = gb * GB
                    for c in range(CH):
                        for rs in range(RS):
                            xo = base + rs * 512 + c * 32
                            wo = base + 2 * 2048 + rs * 512 + c * 32
                            nc.tensor.matmul(
                                out=ph[rs * 32:(rs + 1) * 32, cs:cs + T],
                                lhsT=xwt[:, wo:wo + K],
                                rhs=xwt[:, xo:xo + T],
                                start=(c == 0),
                                stop=(c == CH - 1),
                                tile_position=(0, rs * 32),
                            )
                            nc.tensor.matmul(
                                out=pl[rs * 32:(rs + 1) * 32, cs:cs + T],
                                lhsT=xwt[:, wo:wo + K],
                                rhs=xwt[:, 2048 + xo:2048 + xo + T],
                                start=(c == 0),
                                stop=(c == CH - 1),
                                tile_position=(0, rs * 32),
                            )
                    # pot = ps_hi + ps_lo/64 (one PSUM operand per DVE op)
                    nc.vector.tensor_scalar(
                        out=pot[:, g * T:(g + 1) * T], in0=pl[:, cs:cs + T],
                        scalar1=1.0 / LO, scalar2=None, op0=Op.mult)
                    nc.vector.tensor_tensor(
                        out=pot[:, g * T:(g + 1) * T],
                        in0=pot[:, g * T:(g + 1) * T],
                        in1=ph[:, cs:cs + T], op=Op.add)
                    if g < 12 and (g + 1) % 4 == 0:
                        stage_a(g - 3, g + 1)
                    elif g >= 12:
                        stage_a(g, g + 1)
                    if g == 7:
                        chain(0, 0, G // 2)
                    elif g == 15:
                        chain(1, G // 2, G)

    nc.compile()
    return nc


def prep_inputs(rec_field, W):
    """Host-side relayout into the per-core packed fp16 DMA layout."""
    rec_field = np.asarray(rec_field, dtype=np.float32)
    W = np.asarray(W, dtype=np.float32)
    xr = rec_field[:, 0].transpose(1, 2, 0)            # (RF, L, T) f32
    xh = xr.astype(np.float16)
    xl = ((xr - xh.astype(np.float32)) * LO).astype(np.float16)
    wr = W[:, :, 0].transpose(0, 2, 1).astype(np.float16)   # (RF, L, K)

    def lay(a, J):
        # (RF, L, J) -> (d, p, g, rs*ch*J) with l = c*128 + p
        a6 = a.reshape(NCORES, G, RS, CH, 128, J)
        return a6.transpose(0, 4, 1, 2, 3, 5).reshape(NCORES, 128, G, RS * CH * J)

    H = lay(xh, T)
    Lo = lay(xl, T)
    Wl = lay(wr, K)
    blk = np.concatenate([H, Lo, Wl], axis=3)          # (d, p, g, 3*2048)
    return np.ascontiguousarray(blk.reshape(NCORES, 128, G * GB))


def make_in_maps(rec_field, W):
    xwh = prep_inputs(rec_field, W)
    iota = np.ascontiguousarray(
        np.tile(np.arange(T, dtype=np.float32), (128, 1)))
    return [{"iota_t": iota, "xw": xwh[d]} for d in range(NCORES)]


def assemble_output(results):
    """results: per-core dicts with 'out' (128, 512) -> full (T,1,K,RF)."""
    out_full = np.zeros((T, 1, K, RF), np.float32)
    for d in range(NCORES):
        o = np.asarray(results[d]["out"]).reshape(RS, K, G, T)
        o = o.transpose(3, 1, 2, 0).reshape(T, K, G * RS)   # (t, k, b=g*4+rs)
        out_full[:, 0, :, d * (G * RS):(d + 1) * (G * RS)] = o
    return out_full


def get_nc():
    if "nc" not in _CACHE:
        _CACHE["nc"] = build()
    return _CACHE["nc"]


def kernel(rec_field, W, reward=None, **_unused):
    nc = get_nc()
    in_maps = make_in_maps(rec_field, W)
    res = bass_utils.run_bass_kernel_spmd(nc, in_maps, core_ids=list(range(NCORES)))
    return assemble_output(res.results)


# revision 9
# speedup vs baseline: 1.1263x; 1.0091x over previous
"""Trainium2 Bass kernel for nn_Column1_20298015441326 (topk_masking).

Reference computation (per branch r of RF=512, fully independent):
  pot[r,t,k] = sum_l rec_field[t,0,r,l] * W[r,k,0,l]      (T=32, K=32, L=2048)
  thr = pot * (pot > 20);  spikes = sign(thr)
  kWTA top-4 winner mask per branch (SpykeTorch get_k_winners semantics,
  ties broken by lower feature index), out = spikes * mask, -> (T,1,K,RF).

Sharding: branch axis across 8 cores (64 branches/core), no cross-core comms.

v2 (precision-split inputs, DMA-roofline focused):
  The kernel is memory-bound; fp32 traffic was 33.8 MB/core (~78 us at the
  435 GB/s DMA cap). W is sent as fp16 (validated end-to-end: rel err 0.0096
  vs the 2e-2 budget) and x as an exact-ish fp16 pair xh + xl/64 (keeps x at
  ~2^-22 relative, same bytes as fp32), cutting traffic to 25.2 MB/core and
  making every matmul a full-rate 16-bit pass (fp32 matmuls cost 4 cycles/row
  on the PE; fp16 costs 1).

Per-core device layout:
  branches b = g*4 + rs  (g in [0,16) groups, rs in [0,4) col-tiles)
  xw dram (128, G*6144) fp16: per group block of 6144 cols = [xh|xl|w],
  each 2048 = rs*512 + c*32 + (t|k), partition p = contraction lane
  (l = c*128 + p). Transfers slice contiguous column ranges so every DMA
  descriptor is a 12-24 KB contiguous run per partition (hits the 435 GB/s
  aggregate cap at ~610ns/16KB/engine). Taper 1,1,2,...,2,1,1 groups: small
  head so the PE starts early, small tail so the last group computes early.
  PE per (g,rs,c): two fp16 matmuls (hi, lo) accumulate into separate PSUM
  tiles (8 persistent tiles, one bank each; no recycling).
  pot = ps_hi + ps_lo/64 on DVE directly from PSUM (no scalar.copy -> no ACT
  table-load DMAs competing with the input stream).
  Post-processing on DVE as before, but the transpose/top-4/mask/apply chain
  runs twice (groups 0-7 after g=7, groups 8-15 at the end) so only half the
  chain sits on the critical tail; output DMAs go on the gpsimd queue to
  avoid queueing behind input descriptors on the sync queue.
"""

import numpy as np

import concourse.bacc as bacc
import concourse.mybir as mybir
from concourse import bass_utils
from concourse.tile import TileContext

T = 32
K = 32
RF = 512
L = 2048
TH = 20.0
NCORES = 8
G = 16          # branch groups per core
RS = 4          # branches per group (PE col tiles)
CH = 16         # contraction chunks of 128
GB = 3 * 2048   # xw cols per group: [xh | xl | w]
LO = 64.0       # xl scale
# Delivery order of the 16 one-group transfers. The PE processes groups in
# index order 0..15, so placing group 0 late in the delivery makes the whole
# (in-order, data-dependent) PE stream start only once ~8 groups have landed
# -- it then runs as one continuous warm burst that drains the backlog and
# finishes together with the DMA stream, instead of trickling along with the
# DMA in short bursts that keep HAM re-throttling the PE clock to 1.2 GHz.
DELIVERY = [1, 2, 3, 4, 5, 6, 7, 0, 8, 9, 10, 11, 12, 13, 14, 15]
F32 = mybir.dt.float32
F16 = mybir.dt.float16
Ax = mybir.AxisListType
Op = mybir.AluOpType

_CACHE = {}


def build():
    """Build + compile the per-core Bass module (SPMD: same program, 8 cores)."""
    nc = bacc.Bacc("TRN2", target_bir_lowering=False, debug=False, num_devices=NCORES)
    xw = nc.dram_tensor("xw", (128, G * GB), F16, kind="ExternalInput")
    iota_d = nc.dram_tensor("iota_t", (128, T), F32, kind="ExternalInput")
    out = nc.dram_tensor("out", (128, G * T), F32, kind="ExternalOutput")

    with TileContext(nc) as tc:
        with tc.tile_pool(name="io", bufs=11) as io, \
             tc.tile_pool(name="psp", bufs=1, space="PSUM") as psp, \
             tc.tile_pool(name="wk", bufs=1) as wk:
            iota_sb = wk.tile([128, T], F32)
            nc.gpsimd.dma_start(out=iota_sb[:], in_=iota_d[:, :])
            zeros = wk.tile([128, K], F32)
            nc.vector.memset(zeros[:], 0.0)

            pot = wk.tile([128, G * T], F32)
            gt = wk.tile([128, G * T], F32)
            thr = wk.tile([128, G * T], F32)
            sel = wk.tile([128, G * T], F32)
            sel2 = wk.tile([128, G * T], F32)
            # packed (128, 96): [cnt | pad | vals | pad | rowmax | pad] (16 each)
            packed = wk.tile([128, 96], F32)
            nc.vector.memset(packed[:], 0.0)
            first = wk.tile([128, G], F32)
            has = wk.tile([128, G], F32)

            def stage_a(glo, ghi):
                """fire + per-feature stats for groups [glo, ghi)."""
                gn = ghi - glo
                fs = slice(glo * T, ghi * T)
                g3 = gt[:, fs].rearrange("p (g t) -> p g t", t=T)
                t3 = thr[:, fs].rearrange("p (g t) -> p g t", t=T)
                s3 = sel[:, fs].rearrange("p (g t) -> p g t", t=T)
                s23 = sel2[:, fs].rearrange("p (g t) -> p g t", t=T)
                gsl = slice(glo, ghi)
                nc.vector.tensor_scalar(
                    out=gt[:, fs], in0=pot[:, fs], scalar1=TH, scalar2=None,
                    op0=Op.is_gt)
                nc.vector.tensor_tensor(
                    out=thr[:, fs], in0=pot[:, fs], in1=gt[:, fs], op=Op.mult)
                cnt = packed[:, glo:ghi]
                nc.vector.reduce_sum(out=cnt, in_=g3, axis=Ax.X)
                # first spike time: min(32 - cnt, 31)
                nc.vector.tensor_scalar(
                    out=first[:, gsl], in0=cnt, scalar1=32.0, scalar2=-1.0,
                    op0=Op.subtract, op1=Op.mult)
                nc.vector.tensor_scalar(
                    out=first[:, gsl], in0=first[:, gsl], scalar1=31.0,
                    scalar2=None, op0=Op.min)
                # vals_at_first = sum_t thr * (iota_t == first)
                nc.vector.tensor_tensor(
                    out=s3,
                    in0=iota_sb[:, None, :].to_broadcast([128, gn, T]),
                    in1=first[:, gsl, None].to_broadcast([128, gn, T]),
                    op=Op.is_equal)
                nc.vector.tensor_tensor(out=s23, in0=s3, in1=t3, op=Op.mult)
                vals = packed[:, 32 + glo:32 + ghi]
                nc.vector.reduce_sum(out=vals, in_=s23, axis=Ax.X)
                # rowmax = vals * (cnt > 0)
                nc.vector.tensor_scalar(
                    out=has[:, gsl], in0=cnt, scalar1=0.0, scalar2=None,
                    op0=Op.is_gt)
                nc.vector.tensor_tensor(
                    out=packed[:, 64 + glo:64 + ghi], in0=vals, in1=has[:, gsl],
                    op=Op.mult)

            def chain(tag, glo, ghi):
                """transpose + per-branch top-4 + mask + apply + store for
                groups [glo, ghi). Runs on the full packed tile (stale
                columns produce garbage in unused partitions); only the
                [glo, ghi) output columns are written out."""
                # 32x32 block transpose: -> [p=(rs,g), free=k] per 32-block
                tp = wk.tile([128, 96], F32, name=f"tp{tag}")
                nc.vector.transpose(out=tp[:], in_=packed[:])
                cntT = tp[:, 0:32]
                valsT = tp[:, 32:64]
                rowmaxT = tp[:, 64:96]

                # per-branch v = 32 * max_k rowmax;  total = cnt * (vals + v)
                vmax = wk.tile([128, 1], F32, name=f"vmax{tag}")
                nc.vector.reduce_max(out=vmax[:], in_=rowmaxT, axis=Ax.X)
                v32 = wk.tile([128, 1], F32, name=f"v32{tag}")
                nc.vector.tensor_scalar(
                    out=v32[:], in0=vmax[:], scalar1=32.0, scalar2=None,
                    op0=Op.mult)
                tot2 = wk.tile([128, K], F32, name=f"tot2{tag}")
                nc.vector.scalar_tensor_tensor(
                    out=tot2[:], in0=valsT, scalar=v32[:], in1=cntT,
                    op0=Op.add, op1=Op.mult)

                # top-4 with stable (lower index first) tie-break:
                # m4c = max(4th largest, tiny); keep (tot > m4c) plus the first
                # (4 - #gt) entries equal to m4c. The tiny clamp makes the m4=0
                # case (fewer than 4 positive totals) select exactly the
                # positives, since no total equals the clamp value.
                m8 = wk.tile([128, 8], F32, name=f"m8{tag}")
                nc.vector.max(out=m8[:], in_=tot2[:])
                m4c = wk.tile([128, 1], F32, name=f"m4c{tag}")
                nc.vector.tensor_scalar(
                    out=m4c[:], in0=m8[:, 3:4], scalar1=1e-30, scalar2=None,
                    op0=Op.max)
                sg = wk.tile([128, K], F32, name=f"sg{tag}")
                eq = wk.tile([128, K], F32, name=f"eq{tag}")
                nc.vector.tensor_scalar(
                    out=sg[:], in0=tot2[:], scalar1=m4c[:], scalar2=None,
                    op0=Op.is_gt)
                nc.vector.tensor_scalar(
                    out=eq[:], in0=tot2[:], scalar1=m4c[:], scalar2=None,
                    op0=Op.is_equal)
                ng = wk.tile([128, 1], F32, name=f"ng{tag}")
                nc.vector.reduce_sum(out=ng[:], in_=sg[:], axis=Ax.X)
                need = wk.tile([128, 1], F32, name=f"need{tag}")
                nc.vector.tensor_scalar(
                    out=need[:], in0=ng[:], scalar1=4.0, scalar2=-1.0,
                    op0=Op.subtract, op1=Op.mult)
                incl = wk.tile([128, K], F32, name=f"incl{tag}")
                nc.vector.tensor_tensor_scan(
                    out=incl[:], data0=eq[:], data1=zeros[:], initial=0.0,
                    op0=Op.add, op1=Op.add)
                # eq-element selected iff inclusive-rank <= need
                seleq = wk.tile([128, K], F32, name=f"seleq{tag}")
                nc.vector.tensor_scalar(
                    out=seleq[:], in0=incl[:], scalar1=need[:], scalar2=None,
                    op0=Op.is_le)
                eqs = wk.tile([128, K], F32, name=f"eqs{tag}")
                nc.vector.tensor_tensor(
                    out=eqs[:], in0=eq[:], in1=seleq[:], op=Op.mult)
                maskT = wk.tile([128, K], F32, name=f"maskT{tag}")
                nc.vector.tensor_tensor(
                    out=maskT[:], in0=sg[:], in1=eqs[:], op=Op.add)

                # transpose mask back to [p=(rs,k), free=g], apply, store
                maskA = wk.tile([128, K], F32, name=f"maskA{tag}")
                nc.vector.transpose(out=maskA[:], in_=maskT[:])
                gn = ghi - glo
                fs = slice(glo * T, ghi * T)
                outt = wk.tile([128, gn * T], F32, name=f"outt{tag}")
                o3 = outt[:].rearrange("p (g t) -> p g t", t=T)
                g3 = gt[:, fs].rearrange("p (g t) -> p g t", t=T)
                nc.vector.tensor_tensor(
                    out=o3, in0=g3,
                    in1=maskA[:, glo:ghi, None].to_broadcast([128, gn, T]),
                    op=Op.mult)
                nc.gpsimd.dma_start(out=out[:, fs], in_=outt[:])

            # 8 persistent PSUM tiles (one bank each): hi/lo accumulators for
            # group g%4 at column slice (g//4)*32 (consecutive groups rotate
            # across banks so a group's combine has 4 groups of slack before
            # its bank columns are needed again). No slot recycling -> no
            # release waits on the PE chain.
            ph4 = [psp.tile([128, 4 * T], F32, tag=f"ph{j}", name=f"ph{j}")
                   for j in range(4)]
            pl4 = [psp.tile([128, 4 * T], F32, tag=f"pl{j}", name=f"pl{j}")
                   for j in range(4)]

            # Issue every input transfer up front, in delivery order (sync
            # queue; pool-slot recycling gates the last few at runtime).
            gtile = {}
            for g in DELIVERY:
                xwt = io.tile([128, GB], F16, tag="xw")
                nc.sync.dma_start(out=xwt[:], in_=xw[:, g * GB:(g + 1) * GB])
                gtile[g] = xwt

            for g in range(G):
                if True:
                    xwt = gtile[g]
                    ph = ph4[g % 4]
                    pl = pl4[g % 4]
                    cs = (g // 4) * T
                    base = 0
                    for c in range(CH):
                        for rs in range(RS):
                            xo = base + rs * 512 + c * 32
                            wo = base + 2 * 2048 + rs * 512 + c * 32
                            nc.tensor.matmul(
                                out=ph[rs * 32:(rs + 1) * 32, cs:cs + T],
                                lhsT=xwt[:, wo:wo + K],
                                rhs=xwt[:, xo:xo + T],
                                start=(c == 0),
                                stop=(c == CH - 1),
                                tile_position=(0, rs * 32),
                            )
                            nc.tensor.matmul(
                                out=pl[rs * 32:(rs + 1) * 32, cs:cs + T],
                                lhsT=xwt[:, wo:wo + K],
                                rhs=xwt[:, 2048 + xo:2048 + xo + T],
                                start=(c == 0),
                                stop=(c == CH - 1),
                                tile_position=(0, rs * 32),
                            )
                    # pot = ps_hi + ps_lo/64 (one PSUM operand per DVE op)
                    nc.vector.tensor_scalar(
                        out=pot[:, g * T:(g + 1) * T], in0=pl[:, cs:cs + T],
                        scalar1=1.0 / LO, scalar2=None, op0=Op.mult)
                    nc.vector.tensor_tensor(
                        out=pot[:, g * T:(g + 1) * T],
                        in0=pot[:, g * T:(g + 1) * T],
                        in1=ph[:, cs:cs + T], op=Op.add)
                    if g < 12 and (g + 1) % 4 == 0:
                        stage_a(g - 3, g + 1)
                    elif g >= 12:
                        stage_a(g, g + 1)
                    if g == 7:
                        chain(0, 0, G // 2)
                    elif g == 15:
                        chain(1, G // 2, G)

    nc.compile()
    return nc


def prep_inputs(rec_field, W):
    """Host-side relayout into the per-core packed fp16 DMA layout."""
    rec_field = np.asarray(rec_field, dtype=np.float32)
    W = np.asarray(W, dtype=np.float32)
    xr = rec_field[:, 0].transpose(1, 2, 0)            # (RF, L, T) f32
    xh = xr.astype(np.float16)
    xl = ((xr - xh.astype(np.float32)) * LO).astype(np.float16)
    wr = W[:, :, 0].transpose(0, 2, 1).astype(np.float16)   # (RF, L, K)

    def lay(a, J):
        # (RF, L, J) -> (d, p, g, rs*ch*J) with l = c*128 + p
        a6 = a.reshape(NCORES, G, RS, CH, 128, J)
        return a6.transpose(0, 4, 1, 2, 3, 5).reshape(NCORES, 128, G, RS * CH * J)

    H = lay(xh, T)
    Lo = lay(xl, T)
    Wl = lay(wr, K)
    blk = np.concatenate([H, Lo, Wl], axis=3)          # (d, p, g, 3*2048)
    return np.ascontiguousarray(blk.reshape(NCORES, 128, G * GB))


def make_in_maps(rec_field, W):
    xwh = prep_inputs(rec_field, W)
    iota = np.ascontiguousarray(
        np.tile(np.arange(T, dtype=np.float32), (128, 1)))
    return [{"iota_t": iota, "xw": xwh[d]} for d in range(NCORES)]


def assemble_output(results):
    """results: per-core dicts with 'out' (128, 512) -> full (T,1,K,RF)."""
    out_full = np.zeros((T, 1, K, RF), np.float32)
    for d in range(NCORES):
        o = np.asarray(results[d]["out"]).reshape(RS, K, G, T)
        o = o.transpose(3, 1, 2, 0).reshape(T, K, G * RS)   # (t, k, b=g*4+rs)
        out_full[:, 0, :, d * (G * RS):(d + 1) * (G * RS)] = o
    return out_full


def get_nc():
    if "nc" not in _CACHE:
        _CACHE["nc"] = build()
    return _CACHE["nc"]


def kernel(rec_field, W, reward=None, **_unused):
    nc = get_nc()
    in_maps = make_in_maps(rec_field, W)
    res = bass_utils.run_bass_kernel_spmd(nc, in_maps, core_ids=list(range(NCORES)))
    return assemble_output(res.results)
